# revision 28
# baseline (speedup 1.0000x reference)
"""Causal single-head self-attention on 8 trn2 NeuronCores.

Problem: x [4, 4096, 1024] fp32, w_q/w_k/w_v [1024, 64] fp32.
  q,k,v = x @ w_{q,k,v};  y = softmax(causal(q k^T) / 8) v   -> [4, 4096, 64]

Sharding: 8 cores = 4 batches x 2 query-parity shards. Core c handles
batch b = c//2 and the query rows  h::2  (h = c%2). Interleaving the
query rows by parity makes every core's causal structure identical, so
one SPMD program serves all 8 cores; the h-dependence is folded into a
per-core causal-mask input tensor.

Per-core kernel (Bass/Tile):
  - DMA x[b] in s-tiles of 128 rows, PE-transpose to x^T (E on partitions)
  - kv^T = [w_k|w_v]^T x^T  (fp32r matmuls, E contracted in 8 chunks)
  - q^T from the core's parity columns of x^T
  - v tiles ([128k, 65] with a ones column) by PE-transposing kv^T rows 64:128
  - per local q-block of 512: scores^T = k^T-chunk^T q^T -> exp (ACT, fp32r out)
    -> diag tiles masked (DVE mul with mask input) -> y^T accumulated in PSUM
    via lhsT=v_aug (ones column gives the softmax denominator as row 64)
  - y^T -> PE transpose -> divide by denominator -> DMA out
"""
import sys

sys.path.insert(0, "/opt/trn_rl_repo")

import numpy as np

import concourse.bass as bass
import concourse.mybir as mybir
from concourse import bacc
from concourse.tile import TileContext
from concourse.masks import make_identity

F32 = mybir.dt.float32
F32R = mybir.dt.float32r

B, S, E, D = 4, 4096, 1024, 64
NCORES = 8
SL = S // 2          # local q rows per core (parity shard)
NE = E // 128        # 8 E-chunks
NST = S // 128       # 32 s-tiles of x
NKT = S // 128       # 32 k-tiles
QB = 512             # local q-block size (spans 1024 global rows)
NQB = SL // QB       # 4 local q-blocks
NDIAG = 8            # diagonal k-tiles per q-block (1024 global rows / 128)


def build_nc(iters=1):
    from contextlib import ExitStack

    nc = bacc.Bacc(trn_type="TRN2", num_devices=NCORES)
    xb = nc.declare_dram_parameter("xb", [SL, E], F32, isOutput=False)
    wkv = nc.declare_dram_parameter("wkv", [E, 128], F32, isOutput=False)
    wq = nc.declare_dram_parameter("wq", [E, D], F32, isOutput=False)
    masks = nc.declare_dram_parameter("masks", [NDIAG, 128, QB], F32, isOutput=False)
    y_out = nc.declare_dram_parameter("y", [SL, D], F32, isOutput=True)
    kv_snd = nc.dram_tensor("kv_snd", [128, SL], F32)
    kv_gat = nc.dram_tensor("kv_gat", [2, 128, SL], F32)
    pair_groups = [[2 * p, 2 * p + 1] for p in range(NCORES // 2)]

    with TileContext(nc) as tc:
        with tc.tile_pool(name="singles", bufs=1) as singles, \
             tc.tile_pool(name="big", bufs=1) as big, \
             tc.tile_pool(name="work", bufs=1) as work:
            ident = singles.tile([128, 128], F32)
            make_identity(nc, ident)
            ident_r = singles.tile([128, 128], F32R)
            nc.vector.tensor_copy(out=ident_r, in_=ident)

            # weights as [128, chunk, cols] fp32r
            wkv_sb = singles.tile([128, NE, 128], F32R)
            nc.sync.dma_start(
                out=wkv_sb,
                in_=wkv.rearrange("(e p) c -> p e c", p=128).bitcast(F32R),
            )
            wq_sb = singles.tile([128, NE, D], F32R)
            nc.sync.dma_start(
                out=wq_sb,
                in_=wq.rearrange("(e p) c -> p e c", p=128).bitcast(F32R),
            )
            masks_sb = singles.tile([128, NDIAG, QB], F32)
            nc.sync.dma_start(
                out=masks_sb,
                in_=masks.rearrange("r p c -> p r c"),
            )

            kvT = big.tile([128, S], F32R)      # rows 0:64 = k^T, 64:128 = v^T
            qT = big.tile([64, SL], F32R)
            # replicas on partitions 64:128 so score matmuls (K=64) can be
            # row-packed two-at-a-time into disjoint PE row groups
            kT_hi = big.tile([128, S], F32R)
            qT_hi = big.tile([128, SL], F32R)

            hint = (
                mybir.EngineType.PE,
                mybir.EngineType.DVE,
                mybir.EngineType.Activation,
                mybir.EngineType.SP,
            )
            loop_ctx = ExitStack()
            if iters > 1:
                loop_ctx.enter_context(tc.For_i(0, iters, 1, hint_engines=hint))
            # ------- phase 1: x^T, projections of the core's own rows -------
            with tc.tile_pool(name="ph1ps", bufs=1, space="PSUM") as ps1, \
                 tc.tile_pool(name="ph1sb", bufs=1) as sb1:
                kvT_own = sb1.tile([128, SL], F32, name="kvT_own")
                xt = [
                    sb1.tile([128, SL], F32R, tag=f"xt{e}", name=f"xt{e}")
                    for e in range(NE)
                ]
                for st in range(SL // 128):
                    x_stage = sb1.tile([128, E], F32R, tag="xstage", bufs=3,
                                       name="x_stage")
                    nc.sync.dma_start(
                        out=x_stage,
                        in_=xb[st * 128:(st + 1) * 128, :].bitcast(F32R),
                    )
                    for e in range(NE):
                        ptr = ps1.tile([128, 128], F32R, tag="ptr", bufs=4,
                                       name="ptr")
                        nc.tensor.transpose(
                            ptr, x_stage[:, e * 128:(e + 1) * 128], ident_r
                        )
                        nc.vector.tensor_copy(
                            out=xt[e][:, st * 128:(st + 1) * 128], in_=ptr
                        )
                for sb_i in range(SL // 512):
                    sl = slice(sb_i * 512, (sb_i + 1) * 512)
                    pkv = ps1.tile([128, 512], F32, tag="pkv", bufs=2,
                                   name="pkv")
                    for e in range(NE):
                        nc.tensor.matmul(
                            pkv, wkv_sb[:, e, :], xt[e][:, sl],
                            start=(e == 0), stop=(e == NE - 1),
                        )
                    nc.vector.tensor_copy(out=kvT_own[:, sl], in_=pkv)
                for sb_i in range(SL // 512):
                    sl = slice(sb_i * 512, (sb_i + 1) * 512)
                    pq = ps1.tile([64, 512], F32, tag="pq", bufs=2,
                                  name="pq")
                    for e in range(NE):
                        nc.tensor.matmul(
                            pq, wq_sb[:, e, :], xt[e][:, sl],
                            start=(e == 0), stop=(e == NE - 1),
                        )
                    nc.vector.tensor_copy(out=qT[:, sl], in_=pq)
                # q replica on partitions 64:128 for row-packed scores
                nc.sync.dma_start(out=qT_hi[64:128, :], in_=qT[:, :])
                # send own k^T/v^T half
                nc.sync.dma_start(out=kv_snd[:, :], in_=kvT_own)

            if iters > 1:
                # the AllGather cannot live inside a HW loop; close the
                # phase-1 loop, exchange once, and loop phase 2 separately.
                loop_ctx.close()
            # exchange k^T/v^T halves inside each batch pair
            nc.gpsimd.collective_compute(
                "AllGather", mybir.AluOpType.bypass,
                replica_groups=pair_groups,
                ins=[kv_snd[:, :]], outs=[kv_gat[:, :, :]],
            )
            if iters > 1:
                loop_ctx.enter_context(tc.For_i(0, iters, 1, hint_engines=hint))
            for g in range(2):
                nc.sync.dma_start(
                    out=kvT[:, g * SL:(g + 1) * SL],
                    in_=kv_gat[g].bitcast(F32R),
                )
                # k^T replica on partitions 64:128 for row-packed scores
                nc.sync.dma_start(
                    out=kT_hi[64:128, g * SL:(g + 1) * SL],
                    in_=kv_gat[g][0:64, :].bitcast(F32R),
                )

            # ---------------- phase 2: v tiles + attention ----------------
            with tc.tile_pool(name="ph2ps", bufs=1, space="PSUM") as ps2, \
                 tc.tile_pool(name="ph2sb", bufs=1) as sb2:
                v_aug = work.tile([128, NKT, 65], F32R)
                nc.vector.memset(v_aug.bitcast(F32), 1.0)
                for kt in range(NKT):
                    pvt = ps2.tile([128, 64], F32R, tag="ptile", bufs=2,
                                   name="pvt")
                    nc.tensor.transpose(
                        pvt, kvT[64:128, kt * 128:(kt + 1) * 128],
                        ident_r[64:128, 64:128],
                    )
                    nc.vector.tensor_copy(out=v_aug[:, kt, 0:64], in_=pvt)

                for j in range(NQB):
                    qsl = slice(j * QB, (j + 1) * QB)
                    y_ps = ps2.tile([65, QB], F32, tag=f"y{j % 2}", bufs=1,
                                    name="y_ps")
                    # gathered key order: tiles 0:16 = even global rows,
                    # 16:32 = odd (pair ranks 2b, 2b+1). Pair tile t (even
                    # half, PE rows 0:64) with tile 16+t (odd half, rows
                    # 64:128) so the two K=64 score matmuls run concurrently.
                    pairs = [(t, 16 + t, None, None) for t in range(4 * j)]
                    pairs += [(4 * j + r, 16 + 4 * j + r, r, 4 + r)
                              for r in range(4)]
                    n_av = 2 * len(pairs)
                    iav = 0
                    for (kta, ktb, ra, rb) in pairs:
                        sa = ps2.tile([128, QB], F32, tag="s", bufs=4,
                                      name="sa")
                        sb_ = ps2.tile([128, QB], F32, tag="s", bufs=4,
                                       name="sb_")
                        nc.tensor.matmul(
                            sa, kvT[0:64, kta * 128:(kta + 1) * 128],
                            qT[:, qsl], start=True, stop=True,
                        )
                        nc.tensor.matmul(
                            sb_, kT_hi[64:128, ktb * 128:(ktb + 1) * 128],
                            qT_hi[64:128, qsl], start=True, stop=True,
                        )
                        for (kt, r, s_ps) in ((kta, ra, sa), (ktb, rb, sb_)):
                            eT = sb2.tile([128, QB], F32R, tag="eT", bufs=6,
                                          name="eT")
                            if r is None:
                                nc.scalar.activation(
                                    out=eT, in_=s_ps,
                                    func=mybir.ActivationFunctionType.Exp,
                                    scale=0.125,
                                )
                            else:
                                ef = sb2.tile([128, QB], F32, tag="ef",
                                              bufs=3, name="ef")
                                nc.scalar.activation(
                                    out=ef, in_=s_ps,
                                    func=mybir.ActivationFunctionType.Exp,
                                    scale=0.125,
                                )
                                nc.vector.tensor_mul(
                                    eT, ef, masks_sb[:, r, :]
                                )
                            nc.tensor.matmul(
                                y_ps, v_aug[:, kt, :], eT,
                                start=(iav == 0), stop=(iav == n_av - 1),
                            )
                            iav += 1
                    # finalize q-block
                    ysb = sb2.tile([65, QB], F32, tag="ysb", bufs=2,
                                   name="ysb")
                    nc.vector.tensor_copy(out=ysb, in_=y_ps)
                    for qq in range(QB // 128):
                        pyt = ps2.tile([128, 65], F32, tag="ptile", bufs=2,
                                       name="pyt")
                        nc.tensor.transpose(
                            pyt, ysb[:, qq * 128:(qq + 1) * 128],
                            ident[0:65, 0:65],
                        )
                        yt = sb2.tile([128, 65], F32, tag="yt", bufs=2,
                                      name="yt")
                        nc.vector.tensor_copy(out=yt, in_=pyt)
                        rec = sb2.tile([128, 1], F32, tag="rec", bufs=2,
                                       name="rec")
                        nc.vector.reciprocal(rec, yt[:, 64:65])
                        yo = sb2.tile([128, 64], F32, tag="yo", bufs=2,
                                      name="yo")
                        nc.vector.tensor_scalar_mul(yo, yt[:, 0:64], rec)
                        row = j * QB + qq * 128
                        nc.sync.dma_start(
                            out=y_out[row:row + 128, :], in_=yo
                        )
            loop_ctx.close()
    nc.finalize()
    return nc


class _Runner:
    """Compile once; re-execute the sharded program with cached jit.

    Replicates concourse.bass2jax.run_bass_via_pjrt's multi-core path but
    keeps the jitted function (and optionally device-resident inputs) across
    calls instead of re-tracing per invocation.
    """

    def __init__(self, nc):
        import jax
        from jax.sharding import Mesh, PartitionSpec
        from jax.experimental.shard_map import shard_map
        from concourse import bass2jax, mybir as _mb

        bass2jax.install_neuronx_cc_hook()
        self.nc = nc
        self._jax = jax
        self._bass2jax = bass2jax

        partition_name = (
            nc.partition_id_tensor.name if nc.partition_id_tensor else None
        )
        in_names, out_names, out_avals, zero_shapes = [], [], [], []
        for alloc in nc.m.functions[0].allocations:
            if not isinstance(alloc, _mb.MemoryLocationSet):
                continue
            name = alloc.memorylocations[0].name
            if alloc.kind == "ExternalInput":
                if name != partition_name:
                    in_names.append(name)
            elif alloc.kind == "ExternalOutput":
                shape = tuple(alloc.tensor_shape)
                dtype = _mb.dt.np(alloc.dtype)
                out_names.append(name)
                out_avals.append(jax.core.ShapedArray(shape, dtype))
                zero_shapes.append((shape, dtype))
        self.in_names = list(in_names)
        self.out_names = out_names
        self.zero_shapes = zero_shapes
        n_params = len(in_names)
        n_outs = len(out_avals)
        all_in_names = list(in_names) + list(out_names)
        if partition_name is not None:
            all_in_names.append(partition_name)
        donate = tuple(range(n_params, n_params + n_outs))

        def _body(*args):
            operands = list(args)
            if partition_name is not None:
                operands.append(bass2jax.partition_id_tensor())
            outs = bass2jax._bass_exec_p.bind(
                *operands,
                out_avals=tuple(out_avals),
                in_names=tuple(all_in_names),
                out_names=tuple(out_names),
                lowering_input_output_aliases=(),
                sim_require_finite=True,
                sim_require_nnan=True,
                nc=nc,
            )
            return tuple(outs)

        devices = jax.devices()[:NCORES]
        mesh = Mesh(np.asarray(devices), ("core",))
        in_specs = (PartitionSpec("core"),) * (n_params + n_outs)
        out_specs = (PartitionSpec("core"),) * n_outs
        self.sharded = jax.jit(
            shard_map(_body, mesh=mesh, in_specs=in_specs, out_specs=out_specs,
                      check_rep=False),
            donate_argnums=donate, keep_unused=True,
        )
        self.mesh = mesh
        self.pspec = PartitionSpec("core")

    def put_inputs(self, in_maps):
        """Concat per-core inputs and move to device once."""
        import jax
        from jax.sharding import NamedSharding
        sh = NamedSharding(self.mesh, self.pspec)
        arrs = []
        for name in self.in_names:
            cat = np.concatenate([np.asarray(m[name]) for m in in_maps], axis=0)
            arrs.append(jax.device_put(cat, sh))
        return arrs

    def zeros(self):
        import jax
        from jax.sharding import NamedSharding
        sh = NamedSharding(self.mesh, self.pspec)
        return [
            jax.device_put(np.zeros((NCORES * s[0], *s[1:]), d), sh)
            for (s, d) in self.zero_shapes
        ]

    def run(self, dev_inputs):
        outs = self.sharded(*dev_inputs, *self.zeros())
        return outs

    def results(self, outs):
        out = {}
        for i, name in enumerate(self.out_names):
            a = np.asarray(outs[i])
            out[name] = a.reshape(NCORES, a.shape[0] // NCORES, *a.shape[1:])
        return out


_RUNNER = None


def _get_runner():
    global _RUNNER
    if _RUNNER is None:
        _RUNNER = _Runner(build_nc())
    return _RUNNER


def _make_masks(h: int) -> np.ndarray:
    # Causal masks for the 8 diagonal k-tiles of a q-block, in gathered key
    # order: index r' = kp*4 + r with kp = key parity (0 = rank-2b rows =
    # even global rows, 1 = odd), r = k-subtile within the block's span.
    # key global = 1024j + 256r + 2p + kp ; query global = 1024j + 2c + h.
    # valid iff 256r + 2p + kp <= 2c + h.
    out = np.empty((NDIAG, 128, QB), dtype=np.float32)
    p = np.arange(128)[:, None]
    c = np.arange(QB)[None, :]
    for kp in range(2):
        for r in range(4):
            out[kp * 4 + r] = (256 * r + 2 * p + kp <= 2 * c + h)
    return out


def make_in_maps(x, w_q, w_k, w_v):
    x = np.ascontiguousarray(np.asarray(x, dtype=np.float32))
    w_q = np.asarray(w_q, dtype=np.float32)
    w_k = np.asarray(w_k, dtype=np.float32)
    w_v = np.asarray(w_v, dtype=np.float32)
    wkv = np.ascontiguousarray(np.concatenate([w_k, w_v], axis=1))
    masks = [_make_masks(0), _make_masks(1)]

    in_maps = []
    for c in range(NCORES):
        b, h = c // 2, c % 2
        in_maps.append({
            "xb": np.ascontiguousarray(x[b, h::2]),
            "wkv": wkv,
            "wq": w_q,
            "masks": masks[h],
        })
    return in_maps


def kernel(x, w_q, w_k, w_v):
    runner = _get_runner()
    in_maps = make_in_maps(x, w_q, w_k, w_v)
    dev_inputs = runner.put_inputs(in_maps)
    outs = runner.results(runner.run(dev_inputs))

    y = np.empty((B, S, D), dtype=np.float32)
    for c in range(NCORES):
        b, h = c // 2, c % 2
        y[b, h::2, :] = outs["y"][c]
    return y


# revision 29
# speedup vs baseline: 1.1632x; 1.1632x over previous
"""Causal single-head self-attention on 8 trn2 NeuronCores.

Problem: x [4, 4096, 1024] fp32, w_q/w_k/w_v [1024, 64] fp32.
  q,k,v = x @ w_{q,k,v};  y = softmax(causal(q k^T) / 8) v   -> [4, 4096, 64]

Sharding: 8 cores = 4 batches x 2 query-parity shards. Core c handles
batch b = c//2 and the query rows  h::2  (h = c%2). Interleaving the
query rows by parity makes every core's causal structure identical, so
one SPMD program serves all 8 cores; the h-dependence is folded into a
per-core causal-mask input tensor.

Per-core kernel (Bass/Tile):
  - DMA x[b] in s-tiles of 128 rows, PE-transpose to x^T (E on partitions)
  - kv^T = [w_k|w_v]^T x^T  (fp32r matmuls, E contracted in 8 chunks)
  - q^T from the core's parity columns of x^T
  - v tiles ([128k, 65] with a ones column) by PE-transposing kv^T rows 64:128
  - per local q-block of 512: scores^T = k^T-chunk^T q^T -> exp (ACT, fp32r out)
    -> diag tiles masked (DVE mul with mask input) -> y^T accumulated in PSUM
    via lhsT=v_aug (ones column gives the softmax denominator as row 64)
  - y^T -> PE transpose -> divide by denominator -> DMA out
"""
import sys

sys.path.insert(0, "/opt/trn_rl_repo")

import numpy as np

import concourse.bass as bass
import concourse.mybir as mybir
from concourse import bacc
from concourse.tile import TileContext
from concourse.masks import make_identity

F32 = mybir.dt.float32
F32R = mybir.dt.float32r

B, S, E, D = 4, 4096, 1024, 64
NCORES = 8
SL = S // 2          # local q rows per core (parity shard)
NE = E // 128        # 8 E-chunks
NST = S // 128       # 32 s-tiles of x
NKT = S // 128       # 32 k-tiles
QB = 512             # local q-block size (spans 1024 global rows)
NQB = SL // QB       # 4 local q-blocks
NDIAG = 8            # diagonal k-tiles per q-block (1024 global rows / 128)


def build_nc(iters=1):
    from contextlib import ExitStack

    nc = bacc.Bacc(trn_type="TRN2", num_devices=NCORES)
    xb = nc.declare_dram_parameter("xb", [SL, E], F32, isOutput=False)
    wkv = nc.declare_dram_parameter("wkv", [E, 128], F32, isOutput=False)
    wq = nc.declare_dram_parameter("wq", [E, D], F32, isOutput=False)
    masks = nc.declare_dram_parameter("masks", [NDIAG, 128, QB], F32, isOutput=False)
    y_out = nc.declare_dram_parameter("y", [SL, D], F32, isOutput=True)
    kv_snd = nc.dram_tensor("kv_snd", [128, SL], F32)
    kv_gat = nc.dram_tensor("kv_gat", [2, 128, SL], F32)
    pair_groups = [[2 * p, 2 * p + 1] for p in range(NCORES // 2)]

    with TileContext(nc) as tc:
        with tc.tile_pool(name="singles", bufs=1) as singles, \
             tc.tile_pool(name="big", bufs=1) as big, \
             tc.tile_pool(name="work", bufs=1) as work:
            ident = singles.tile([128, 128], F32)
            make_identity(nc, ident)
            ident_r = singles.tile([128, 128], F32R)
            nc.vector.tensor_copy(out=ident_r, in_=ident)

            # weights as [128, chunk, cols] fp32r
            wkv_sb = singles.tile([128, NE, 128], F32R)
            nc.sync.dma_start(
                out=wkv_sb,
                in_=wkv.rearrange("(e p) c -> p e c", p=128).bitcast(F32R),
            )
            wq_sb = singles.tile([128, NE, D], F32R)
            nc.sync.dma_start(
                out=wq_sb,
                in_=wq.rearrange("(e p) c -> p e c", p=128).bitcast(F32R),
            )
            masks_sb = singles.tile([128, NDIAG, QB], F32)
            nc.sync.dma_start(
                out=masks_sb,
                in_=masks.rearrange("r p c -> p r c"),
            )

            kvT = big.tile([128, S], F32R)      # rows 0:64 = k^T, 64:128 = v^T
            qT = big.tile([64, SL], F32R)

            hint = (
                mybir.EngineType.PE,
                mybir.EngineType.DVE,
                mybir.EngineType.Activation,
                mybir.EngineType.SP,
            )
            loop_ctx = ExitStack()
            if iters > 1:
                loop_ctx.enter_context(tc.For_i(0, iters, 1, hint_engines=hint))
            # ------- phase 1: x^T, projections of the core's own rows -------
            with tc.tile_pool(name="ph1ps", bufs=1, space="PSUM") as ps1, \
                 tc.tile_pool(name="ph1sb", bufs=1) as sb1:
                kvT_own = sb1.tile([128, SL], F32, name="kvT_own")
                xt = [
                    sb1.tile([128, SL], F32R, tag=f"xt{e}", name=f"xt{e}")
                    for e in range(NE)
                ]
                for st in range(SL // 128):
                    x_stage = sb1.tile([128, E], F32, tag="xstage", bufs=3,
                                       name="x_stage")
                    nc.sync.dma_start(
                        out=x_stage,
                        in_=xb[st * 128:(st + 1) * 128, :],
                    )
                    for e in range(NE):
                        ptr = ps1.tile([128, 128], F32, tag="ptr", bufs=4,
                                       name="ptr")
                        nc.tensor.transpose(
                            ptr, x_stage[:, e * 128:(e + 1) * 128], ident
                        )
                        nc.vector.tensor_copy(
                            out=xt[e][:, st * 128:(st + 1) * 128], in_=ptr
                        )
                for sb_i in range(SL // 512):
                    sl = slice(sb_i * 512, (sb_i + 1) * 512)
                    pkv = ps1.tile([128, 512], F32, tag="pkv", bufs=2,
                                   name="pkv")
                    for e in range(NE):
                        nc.tensor.matmul(
                            pkv, wkv_sb[:, e, :], xt[e][:, sl],
                            start=(e == 0), stop=(e == NE - 1),
                        )
                    nc.vector.tensor_copy(out=kvT_own[:, sl], in_=pkv)
                for sb_i in range(SL // 512):
                    sl = slice(sb_i * 512, (sb_i + 1) * 512)
                    pq = ps1.tile([64, 512], F32, tag="pq", bufs=2,
                                  name="pq")
                    for e in range(NE):
                        nc.tensor.matmul(
                            pq, wq_sb[:, e, :], xt[e][:, sl],
                            start=(e == 0), stop=(e == NE - 1),
                        )
                    nc.vector.tensor_copy(out=qT[:, sl], in_=pq)
                # send own k^T/v^T half
                nc.sync.dma_start(out=kv_snd[:, :], in_=kvT_own)

            if iters > 1:
                # the AllGather cannot live inside a HW loop; close the
                # phase-1 loop, exchange once, and loop phase 2 separately.
                loop_ctx.close()
            # exchange k^T/v^T halves inside each batch pair
            nc.gpsimd.collective_compute(
                "AllGather", mybir.AluOpType.bypass,
                replica_groups=pair_groups,
                ins=[kv_snd[:, :]], outs=[kv_gat[:, :, :]],
            )
            if iters > 1:
                loop_ctx.enter_context(tc.For_i(0, iters, 1, hint_engines=hint))
            for g in range(2):
                nc.sync.dma_start(
                    out=kvT[:, g * SL:(g + 1) * SL],
                    in_=kv_gat[g].bitcast(F32R),
                )

            # ---------------- phase 2: v tiles + attention ----------------
            with tc.tile_pool(name="ph2ps", bufs=1, space="PSUM") as ps2, \
                 tc.tile_pool(name="ph2sb", bufs=1) as sb2:
                v_aug = work.tile([128, NKT, 65], F32R)
                nc.vector.memset(v_aug.bitcast(F32), 1.0)
                for kt in range(NKT):
                    pvt = ps2.tile([128, 64], F32R, tag="pvt", bufs=1,
                                   name="pvt")
                    nc.tensor.transpose(
                        pvt, kvT[64:128, kt * 128:(kt + 1) * 128],
                        ident_r[64:128, 64:128],
                    )
                    nc.vector.tensor_copy(out=v_aug[:, kt, 0:64], in_=pvt)

                for j in range(NQB):
                    qsl = slice(j * QB, (j + 1) * QB)
                    y_ps = ps2.tile([65, QB], F32, tag=f"y{j % 2}", bufs=1,
                                    name="y_ps")
                    # gathered key order: tiles 0:16 = even global rows,
                    # 16:32 = odd global rows (pair ranks 2b, 2b+1)
                    full_tiles = [t for t in range(4 * j)] + \
                                 [16 + t for t in range(4 * j)]
                    diag_tiles = [(4 * j + r, r) for r in range(4)] + \
                                 [(16 + 4 * j + r, 4 + r) for r in range(4)]
                    seq = [(kt, None) for kt in full_tiles] + diag_tiles
                    for i, (kt, r) in enumerate(seq):
                        s_ps = ps2.tile([128, QB], F32, tag="s", bufs=3,
                                        name="s_ps")
                        nc.tensor.matmul(
                            s_ps,
                            kvT[0:64, kt * 128:(kt + 1) * 128],
                            qT[:, qsl],
                            start=True, stop=True,
                        )
                        eT = sb2.tile([128, QB], F32R, tag="eT", bufs=6,
                                      name="eT")
                        if r is None:
                            nc.scalar.activation(
                                out=eT, in_=s_ps,
                                func=mybir.ActivationFunctionType.Exp,
                                scale=0.125,
                            )
                        else:
                            ef = sb2.tile([128, QB], F32, tag="ef", bufs=3,
                                          name="ef")
                            nc.scalar.activation(
                                out=ef, in_=s_ps,
                                func=mybir.ActivationFunctionType.Exp,
                                scale=0.125,
                            )
                            nc.vector.tensor_mul(
                                eT, ef, masks_sb[:, r, :]
                            )
                        nc.tensor.matmul(
                            y_ps, v_aug[:, kt, :], eT,
                            start=(i == 0), stop=(i == len(seq) - 1),
                        )
                    # finalize q-block
                    ysb = sb2.tile([65, QB], F32, tag="ysb", bufs=2,
                                   name="ysb")
                    nc.vector.tensor_copy(out=ysb, in_=y_ps)
                    for qq in range(QB // 128):
                        pyt = ps2.tile([128, 65], F32, tag="pyt", bufs=2,
                                       name="pyt")
                        nc.tensor.transpose(
                            pyt, ysb[:, qq * 128:(qq + 1) * 128],
                            ident[0:65, 0:65],
                        )
                        yt = sb2.tile([128, 65], F32, tag="yt", bufs=2,
                                      name="yt")
                        nc.vector.tensor_copy(out=yt, in_=pyt)
                        rec = sb2.tile([128, 1], F32, tag="rec", bufs=2,
                                       name="rec")
                        nc.vector.reciprocal(rec, yt[:, 64:65])
                        yo = sb2.tile([128, 64], F32, tag="yo", bufs=2,
                                      name="yo")
                        nc.vector.tensor_scalar_mul(yo, yt[:, 0:64], rec)
                        row = j * QB + qq * 128
                        nc.sync.dma_start(
                            out=y_out[row:row + 128, :], in_=yo
                        )
            loop_ctx.close()
    nc.finalize()
    return nc


class _Runner:
    """Compile once; re-execute the sharded program with cached jit.

    Replicates concourse.bass2jax.run_bass_via_pjrt's multi-core path but
    keeps the jitted function (and optionally device-resident inputs) across
    calls instead of re-tracing per invocation.
    """

    def __init__(self, nc):
        import jax
        from jax.sharding import Mesh, PartitionSpec
        from jax.experimental.shard_map import shard_map
        from concourse import bass2jax, mybir as _mb

        bass2jax.install_neuronx_cc_hook()
        self.nc = nc
        self._jax = jax
        self._bass2jax = bass2jax

        partition_name = (
            nc.partition_id_tensor.name if nc.partition_id_tensor else None
        )
        in_names, out_names, out_avals, zero_shapes = [], [], [], []
        for alloc in nc.m.functions[0].allocations:
            if not isinstance(alloc, _mb.MemoryLocationSet):
                continue
            name = alloc.memorylocations[0].name
            if alloc.kind == "ExternalInput":
                if name != partition_name:
                    in_names.append(name)
            elif alloc.kind == "ExternalOutput":
                shape = tuple(alloc.tensor_shape)
                dtype = _mb.dt.np(alloc.dtype)
                out_names.append(name)
                out_avals.append(jax.core.ShapedArray(shape, dtype))
                zero_shapes.append((shape, dtype))
        self.in_names = list(in_names)
        self.out_names = out_names
        self.zero_shapes = zero_shapes
        n_params = len(in_names)
        n_outs = len(out_avals)
        all_in_names = list(in_names) + list(out_names)
        if partition_name is not None:
            all_in_names.append(partition_name)
        donate = tuple(range(n_params, n_params + n_outs))

        def _body(*args):
            operands = list(args)
            if partition_name is not None:
                operands.append(bass2jax.partition_id_tensor())
            outs = bass2jax._bass_exec_p.bind(
                *operands,
                out_avals=tuple(out_avals),
                in_names=tuple(all_in_names),
                out_names=tuple(out_names),
                lowering_input_output_aliases=(),
                sim_require_finite=True,
                sim_require_nnan=True,
                nc=nc,
            )
            return tuple(outs)

        devices = jax.devices()[:NCORES]
        mesh = Mesh(np.asarray(devices), ("core",))
        in_specs = (PartitionSpec("core"),) * (n_params + n_outs)
        out_specs = (PartitionSpec("core"),) * n_outs
        self.sharded = jax.jit(
            shard_map(_body, mesh=mesh, in_specs=in_specs, out_specs=out_specs,
                      check_rep=False),
            donate_argnums=donate, keep_unused=True,
        )
        self.mesh = mesh
        self.pspec = PartitionSpec("core")

    def put_inputs(self, in_maps):
        """Concat per-core inputs and move to device once."""
        import jax
        from jax.sharding import NamedSharding
        sh = NamedSharding(self.mesh, self.pspec)
        arrs = []
        for name in self.in_names:
            cat = np.concatenate([np.asarray(m[name]) for m in in_maps], axis=0)
            arrs.append(jax.device_put(cat, sh))
        return arrs

    def zeros(self):
        import jax
        from jax.sharding import NamedSharding
        sh = NamedSharding(self.mesh, self.pspec)
        return [
            jax.device_put(np.zeros((NCORES * s[0], *s[1:]), d), sh)
            for (s, d) in self.zero_shapes
        ]

    def run(self, dev_inputs):
        outs = self.sharded(*dev_inputs, *self.zeros())
        return outs

    def results(self, outs):
        out = {}
        for i, name in enumerate(self.out_names):
            a = np.asarray(outs[i])
            out[name] = a.reshape(NCORES, a.shape[0] // NCORES, *a.shape[1:])
        return out


_RUNNER = None


def _get_runner():
    global _RUNNER
    if _RUNNER is None:
        _RUNNER = _Runner(build_nc())
    return _RUNNER


def _make_masks(h: int) -> np.ndarray:
    # Causal masks for the 8 diagonal k-tiles of a q-block, in gathered key
    # order: index r' = kp*4 + r with kp = key parity (0 = rank-2b rows =
    # even global rows, 1 = odd), r = k-subtile within the block's span.
    # key global = 1024j + 256r + 2p + kp ; query global = 1024j + 2c + h.
    # valid iff 256r + 2p + kp <= 2c + h.
    out = np.empty((NDIAG, 128, QB), dtype=np.float32)
    p = np.arange(128)[:, None]
    c = np.arange(QB)[None, :]
    for kp in range(2):
        for r in range(4):
            out[kp * 4 + r] = (256 * r + 2 * p + kp <= 2 * c + h)
    return out


def make_in_maps(x, w_q, w_k, w_v):
    x = np.ascontiguousarray(np.asarray(x, dtype=np.float32))
    w_q = np.asarray(w_q, dtype=np.float32)
    w_k = np.asarray(w_k, dtype=np.float32)
    w_v = np.asarray(w_v, dtype=np.float32)
    wkv = np.ascontiguousarray(np.concatenate([w_k, w_v], axis=1))
    masks = [_make_masks(0), _make_masks(1)]

    in_maps = []
    for c in range(NCORES):
        b, h = c // 2, c % 2
        in_maps.append({
            "xb": np.ascontiguousarray(x[b, h::2]),
            "wkv": wkv,
            "wq": w_q,
            "masks": masks[h],
        })
    return in_maps


def kernel(x, w_q, w_k, w_v):
    runner = _get_runner()
    in_maps = make_in_maps(x, w_q, w_k, w_v)
    dev_inputs = runner.put_inputs(in_maps)
    outs = runner.results(runner.run(dev_inputs))

    y = np.empty((B, S, D), dtype=np.float32)
    for c in range(NCORES):
        b, h = c // 2, c % 2
        y[b, h::2, :] = outs["y"][c]
    return y
